# revision 3
# baseline (speedup 1.0000x reference)
"""CPC spatial BCE loss kernel for 8 TRN2 NeuronCores.

Computation: loss = BCE(sigmoid((V1.reshape(N,D) @ V2.reshape(N,D).T) / D), eye(N))
with N=256, D=64*64*64=262144.

Strategy (memory-regime): shard the contraction dim D across the 8 cores
and quantize to fp8e4m3 on the host, so each core streams only 16.8 MB
(the 8 cores together saturate the chip's HBM, so bytes-moved is the
whole game; fp8 quantization error on the scalar loss is ~1e-8 relative,
vs the 2e-2 gate). Each core computes a partial Gram matrix [256, 256]
over its 32768-wide slice of D via fp8 DoubleRow TensorE matmuls
(2 fp8 weights/PE cell = 2x ALU rate, keeping TensorE well under the DMA
cadence) accumulated in fp32 PSUM. The host lays out each core's chunk
d-major and pre-tiled to the exact SBUF tile layout, so every chunk DMA
is one fully-contiguous read with the contraction dim landing on SBUF
partitions -- no on-device transposes or casts.

Device pipeline per chunk: fp8 loads stream on the two HWDGE rings (SP
ring for the f1 tile, ACT ring for the adjacent f2 tile of the same
packed buffer -- together they stream at the per-core share of the chip
HBM ceiling); TensorE consumes the tiles directly with DoubleRow
matmuls (lhsT [128d, 2, 128i], rhs [128d, 2, 256j] -> psum [128i, 256j],
two d-blocks per instruction). Chunk sizes are small at the head (fast
pipeline fill) and tail (short drain after the last DMA).

The partial Gram matrices are summed on the host (the unshard step for a
sum-sharded value) and the final sigmoid+BCE over 256x256 values is a
negligible epilogue done in numpy.
"""

import numpy as np

N = 256
D = 64 * 64 * 64  # 262144
NCORES = 8
DLOC = D // NCORES  # 32768
P = 128  # SBUF partitions
MB = 16  # max d-blocks of 128 per DMA chunk

_built = {}
_last_results = None  # test harness reads profiling info from here


def _install_ntff_hook():
    """Best-effort shim: some images lack antenv.axon_hooks, which
    bass_utils imports when profiling is requested (BASS_TRACE) under
    axon. Provide it + register the ctypes NTFF hook so tracing works;
    degrade silently if any piece is missing."""
    import sys
    import types
    try:
        import antenv.axon_hooks  # noqa: F401
        return
    except Exception:
        pass
    try:
        import antenv
        from trn_agent_boot.trn_boot import _ntff_profile_via_ctypes
        mod = types.ModuleType("antenv.axon_hooks")
        mod._hook = None

        def set_axon_ntff_profile_hook(h, _mod=mod):
            _mod._hook = h

        def get_axon_ntff_profile_hook(_mod=mod):
            return _mod._hook

        mod.set_axon_ntff_profile_hook = set_axon_ntff_profile_hook
        mod.get_axon_ntff_profile_hook = get_axon_ntff_profile_hook
        sys.modules["antenv.axon_hooks"] = mod
        antenv.axon_hooks = mod
        hook = _ntff_profile_via_ctypes("/opt/axon/libaxon_pjrt.so")
        if hook is not None:
            set_axon_ntff_profile_hook(hook)
    except Exception:
        pass


def _f8dt():
    import ml_dtypes
    return np.dtype(ml_dtypes.float8_e4m3)


def _sched(dloc=DLOC, mb=MB):
    """Chunk schedule in 128-d blocks: small head (fill) / tail (drain).

    All chunk sizes are even: DoubleRow consumes d-blocks in pairs and a
    pair may not span a chunk boundary.
    """
    nblocks = dloc // P
    assert nblocks % 2 == 0
    if nblocks <= 4 * mb:
        sched = []
        rem = nblocks
        while rem > 0:
            s = min(4, rem)
            sched.append(s)
            rem -= s
        assert all(s % 2 == 0 for s in sched), sched
        return sched
    sched = [2, 2, 4, 8]
    while sum(sched) + mb <= nblocks - 8:
        sched.append(mb)
    rem = nblocks - sum(sched)
    while rem > 6:
        sched.append(4)
        rem -= 4
    while rem > 0:
        s = min(2, rem)
        sched.append(s)
        rem -= s
    assert sum(sched) == nblocks, (sched, nblocks)
    assert all(s % 2 == 0 for s in sched), sched
    return sched


def _build(dloc=DLOC, mb=MB, bufs=3):
    """Build + bacc-compile the per-core Bass kernel.

    Per-core input ft0: [2*dloc, N] fp8e4m3, host pre-tiled so chunk c
    (covering blocks [b0, b0+cmb) of 128 d-values) holds its f1 tile and
    f2 tile adjacently, each in [P, cmb, N] SBUF tile order (row
    r = p*cmb + nb holds f[:, (b0+nb)*P + p]) -- i.e. each chunk DMA is
    one contiguous read mapping partition p <- d within block.
    Output: out[i, j] = sum_d f1t[d, i] * f2t[d, j]   (partial Gram)
    """
    import concourse.mybir as mybir
    from concourse import bacc
    from concourse.bass import MemorySpace
    from concourse.tile import TileContext

    cdt = mybir.dt.float8e4
    DR = mybir.MatmulPerfMode.DoubleRow

    nc = bacc.Bacc("TRN2", target_bir_lowering=False, debug=False,
                   num_devices=NCORES)
    # Both tensors packed chunk-interleaved [f1_c | f2_c | f1_c+1 ...] so the
    # two concurrent ring reads hit adjacent address regions (uniform HBM
    # channel striping instead of two far-apart colliding streams).
    sched = _sched(dloc, mb)
    ft = nc.dram_tensor("ft0", (2 * dloc, N), cdt, kind="ExternalInput")
    out = nc.dram_tensor("out", (N, N), mybir.dt.float32,
                         kind="ExternalOutput")

    fv = ft.ap()

    with TileContext(nc) as tc:
        with tc.tile_pool(name="psum", bufs=1, space=MemorySpace.PSUM) as psum_pool, \
             tc.tile_pool(name="sbuff", bufs=bufs) as poolf, \
             tc.tile_pool(name="outp", bufs=1) as outpool:
            acc = [psum_pool.tile([P, N], mybir.dt.float32, tag=f"acc{ib}",
                                  name=f"acc{ib}")
                   for ib in range(2)]
            b0 = 0
            r0 = 0
            for c, cmb in enumerate(sched):
                # f1 tile and f2 tile are adjacent in the packed buffer;
                # the two HWDGE rings (SP / ACT) stream them concurrently.
                r1 = fv[r0:r0 + cmb * P]
                r2 = fv[r0 + cmb * P:r0 + 2 * cmb * P]
                tb = poolf.tile([P, 2, mb, N], cdt, tag="tf",
                                name=f"tf_{c}")[:, :, :cmb]
                nc.sync.dma_start(
                    out=tb[:, 0], in_=r1.rearrange("(p nb) i -> p nb i", p=P))
                nc.scalar.dma_start(
                    out=tb[:, 1], in_=r2.rearrange("(p nb) i -> p nb i", p=P))
                last_chunk = c == len(sched) - 1
                if not last_chunk:
                    for nb in range(0, cmb, 2):
                        gb = b0 + nb
                        for ib in range(2):
                            nc.tensor.matmul(
                                acc[ib],
                                tb[:, 0, nb:nb + 2, ib * P:(ib + 1) * P],
                                tb[:, 1, nb:nb + 2, :],
                                start=(gb == 0),
                                stop=False,
                                perf_mode=DR,
                            )
                else:
                    # ib-major in the last chunk: acc[0] finishes first so
                    # its PSUM copy + store overlap acc[1]'s final matmuls.
                    for ib in range(2):
                        for nb in range(0, cmb, 2):
                            nc.tensor.matmul(
                                acc[ib],
                                tb[:, 0, nb:nb + 2, ib * P:(ib + 1) * P],
                                tb[:, 1, nb:nb + 2, :],
                                start=False,
                                stop=(nb == cmb - 2),
                                perf_mode=DR,
                            )
                        o = outpool.tile([P, N], mybir.dt.float32,
                                         tag=f"o{ib}", name=f"o{ib}")
                        nc.vector.tensor_copy(o, acc[ib])
                        nc.sync.dma_start(
                            out=out.ap()[ib * P:(ib + 1) * P, :], in_=o)
                b0 += cmb
                r0 += 2 * cmb * P

    nc.compile()
    return nc


def _get_nc():
    if "nc" not in _built:
        _built["nc"] = _build()
    return _built["nc"]


def _gram_partials(in_maps, trace=False):
    global _last_results
    _install_ntff_hook()
    from concourse.bass_utils import run_bass_kernel_spmd

    nc = _get_nc()
    res = run_bass_kernel_spmd(nc, in_maps, core_ids=list(range(NCORES)),
                               trace=trace)
    _last_results = res
    return [r["out"] for r in res.results]


def _pack_core(f1, f2, k, dloc=DLOC, mb=MB):
    """Pack core k's d-chunks of f1, f2 [N, D] fp32 into one [2*dloc, N]
    fp8e4m3 buffer.

    Chunk c covering blocks [b0, b0+cmb): the f1 tile occupies rows
    [2*b0*P, (2*b0+cmb)*P) and the f2 tile the next cmb*P rows, each in
    [P, cmb, N] SBUF tile order (row p*cmb+nb holds f[:, (b0+nb)*P+p]).
    """
    f8 = _f8dt()
    x1 = np.ascontiguousarray(f1[:, k * dloc:(k + 1) * dloc]).astype(f8)
    x2 = np.ascontiguousarray(f2[:, k * dloc:(k + 1) * dloc]).astype(f8)
    sched = _sched(dloc, mb)
    outbuf = np.empty((2 * dloc, N), dtype=f8)
    b0 = 0
    r0 = 0
    for cmb in sched:
        for x in (x1, x2):
            sl = x[:, b0 * P:(b0 + cmb) * P]              # [N, cmb*P]
            t = sl.reshape(N, cmb, P).transpose(2, 1, 0)  # [P, cmb, N]
            outbuf[r0:r0 + cmb * P] = t.reshape(cmb * P, N)
            r0 += cmb * P
        b0 += cmb
    return {"ft0": outbuf}


def kernel(V1, V2):
    V1 = np.asarray(V1, dtype=np.float32)
    V2 = np.asarray(V2, dtype=np.float32)
    f1 = V1.reshape(N, D)
    f2 = V2.reshape(N, D)

    from concurrent.futures import ThreadPoolExecutor
    with ThreadPoolExecutor(NCORES) as ex:
        in_maps = list(ex.map(lambda k: _pack_core(f1, f2, k), range(NCORES)))
    partials = _gram_partials(in_maps)

    Z = np.zeros((N, N), dtype=np.float64)
    for pmat in partials:
        Z += pmat
    Z /= D

    eps = 1e-12
    p = 1.0 / (1.0 + np.exp(-Z))
    p = np.clip(p, eps, 1.0 - eps)
    lab = np.eye(N, dtype=np.float64)
    loss = -np.mean(lab * np.log(p) + (1.0 - lab) * np.log1p(-p))
    return np.array(loss, dtype=np.float32)


def _selftest_sim():
    """Scaled-down correctness check in CoreSim (no hardware)."""
    from concourse.bass_interp import CoreSim

    dloc, mb = 2048, 4
    nc = _build(dloc=dloc, mb=mb)
    rng = np.random.default_rng(0)
    a = rng.standard_normal((N, dloc)).astype(np.float32)  # [N, dloc] like f1
    b = rng.standard_normal((N, dloc)).astype(np.float32)

    sim = CoreSim(nc)
    for name, arr in _pack_core(a, b, 0, dloc=dloc, mb=mb).items():
        sim.tensor(name)[:] = arr
    sim.simulate()
    got = np.array(sim.tensor("out"))
    f8 = _f8dt()
    aq = a.astype(f8).astype(np.float64)
    bq = b.astype(f8).astype(np.float64)
    want_q = aq @ bq.T
    want = a.astype(np.float64) @ b.astype(np.float64).T
    err_q = np.abs(got - want_q).max() / np.abs(want_q).max()
    err = np.abs(got - want).max() / np.abs(want).max()
    print("selftest rel err vs fp8-exact:", err_q, " vs fp32-exact:", err)
    assert err_q < 1e-5, err_q
    # fp8 quantization noise on individual Gram entries at this small
    # dloc; the final scalar loss at full D is ~1e-8 relative.
    assert err < 6e-2, err
    print("SELFTEST PASSED")


if __name__ == "__main__":
    _selftest_sim()


# revision 4
# speedup vs baseline: 1.0942x; 1.0942x over previous
"""CPC spatial BCE loss kernel for 8 TRN2 NeuronCores.

Computation: loss = BCE(sigmoid((V1.reshape(N,D) @ V2.reshape(N,D).T) / D), eye(N))
with N=256, D=64*64*64=262144.

Strategy (memory-regime): shard the contraction dim D across the 8 cores
and quantize to fp8e4m3 on the host, so each core streams only 16.8 MB
(the 8 cores together saturate the chip's HBM, so bytes-moved is the
whole game; fp8 quantization error on the scalar loss is ~1e-8 relative,
vs the 2e-2 gate). Each core computes a partial Gram matrix [256, 256]
over its 32768-wide slice of D via fp8 DoubleRow TensorE matmuls
(2 fp8 weights/PE cell = 2x ALU rate, keeping TensorE well under the DMA
cadence) accumulated in fp32 PSUM. The host lays out each core's chunk
d-major and pre-tiled to the exact SBUF tile layout, so every chunk DMA
is one fully-contiguous read with the contraction dim landing on SBUF
partitions -- no on-device transposes or casts.

Device pipeline per chunk: fp8 loads stream on the two HWDGE rings (SP
ring for the f1 tile, ACT ring for the adjacent f2 tile of the same
packed buffer -- together they stream at the per-core share of the chip
HBM ceiling); TensorE consumes the tiles directly with DoubleRow
matmuls (lhsT [128d, 2, 128i], rhs [128d, 2, 256j] -> psum [128i, 256j],
two d-blocks per instruction). Chunk sizes are small at the head (fast
pipeline fill) and tail (short drain after the last DMA).

The partial Gram matrices are summed on the host (the unshard step for a
sum-sharded value) and the final sigmoid+BCE over 256x256 values is a
negligible epilogue done in numpy.
"""

import numpy as np

N = 256
D = 64 * 64 * 64  # 262144
NCORES = 8
DLOC = D // NCORES  # 32768
P = 128  # SBUF partitions
MB = 8  # max d-blocks of 128 per DMA chunk

_built = {}
_last_results = None  # test harness reads profiling info from here


def _install_ntff_hook():
    """Best-effort shim: some images lack antenv.axon_hooks, which
    bass_utils imports when profiling is requested (BASS_TRACE) under
    axon. Provide it + register the ctypes NTFF hook so tracing works;
    degrade silently if any piece is missing."""
    import sys
    import types
    try:
        import antenv.axon_hooks  # noqa: F401
        return
    except Exception:
        pass
    try:
        import antenv
        from trn_agent_boot.trn_boot import _ntff_profile_via_ctypes
        mod = types.ModuleType("antenv.axon_hooks")
        mod._hook = None

        def set_axon_ntff_profile_hook(h, _mod=mod):
            _mod._hook = h

        def get_axon_ntff_profile_hook(_mod=mod):
            return _mod._hook

        mod.set_axon_ntff_profile_hook = set_axon_ntff_profile_hook
        mod.get_axon_ntff_profile_hook = get_axon_ntff_profile_hook
        sys.modules["antenv.axon_hooks"] = mod
        antenv.axon_hooks = mod
        hook = _ntff_profile_via_ctypes("/opt/axon/libaxon_pjrt.so")
        if hook is not None:
            set_axon_ntff_profile_hook(hook)
    except Exception:
        pass


def _f8dt():
    import ml_dtypes
    return np.dtype(ml_dtypes.float8_e4m3)


def _sched(dloc=DLOC, mb=MB):
    """Chunk schedule in 128-d blocks: small head (fill) / tail (drain).

    All chunk sizes are even: DoubleRow consumes d-blocks in pairs and a
    pair may not span a chunk boundary.
    """
    nblocks = dloc // P
    assert nblocks % 2 == 0
    if nblocks <= 4 * mb:
        sched = []
        rem = nblocks
        while rem > 0:
            s = min(4, rem)
            sched.append(s)
            rem -= s
        assert all(s % 2 == 0 for s in sched), sched
        return sched
    sched = [2, 2, 4, 8]
    while sum(sched) + mb <= nblocks - 8:
        sched.append(mb)
    rem = nblocks - sum(sched)
    while rem > 6:
        sched.append(4)
        rem -= 4
    while rem > 0:
        s = min(2, rem)
        sched.append(s)
        rem -= s
    assert sum(sched) == nblocks, (sched, nblocks)
    assert all(s % 2 == 0 for s in sched), sched
    return sched


def _build(dloc=DLOC, mb=MB, bufs=12):
    """Build + bacc-compile the per-core Bass kernel.

    Per-core input ft0: [2*dloc, N] fp8e4m3, host pre-tiled so chunk c
    (covering blocks [b0, b0+cmb) of 128 d-values) holds its f1 tile and
    f2 tile adjacently, each in [P, cmb, N] SBUF tile order (row
    r = p*cmb + nb holds f[:, (b0+nb)*P + p]) -- i.e. each chunk DMA is
    one contiguous read mapping partition p <- d within block.
    Output: out[i, j] = sum_d f1t[d, i] * f2t[d, j]   (partial Gram)
    """
    import concourse.mybir as mybir
    from concourse import bacc
    from concourse.bass import MemorySpace
    from concourse.tile import TileContext

    cdt = mybir.dt.float8e4
    DR = mybir.MatmulPerfMode.DoubleRow

    nc = bacc.Bacc("TRN2", target_bir_lowering=False, debug=False,
                   num_devices=NCORES)
    # Both tensors packed chunk-interleaved [f1_c | f2_c | f1_c+1 ...] so the
    # two concurrent ring reads hit adjacent address regions (uniform HBM
    # channel striping instead of two far-apart colliding streams).
    sched = _sched(dloc, mb)
    ft = nc.dram_tensor("ft0", (2 * dloc, N), cdt, kind="ExternalInput")
    out = nc.dram_tensor("out", (N, N), mybir.dt.float32,
                         kind="ExternalOutput")

    fv = ft.ap()

    with TileContext(nc) as tc:
        with tc.tile_pool(name="psum", bufs=1, space=MemorySpace.PSUM) as psum_pool, \
             tc.tile_pool(name="sbuff", bufs=bufs) as poolf, \
             tc.tile_pool(name="outp", bufs=1) as outpool:
            acc = [psum_pool.tile([P, N], mybir.dt.float32, tag=f"acc{ib}",
                                  name=f"acc{ib}")
                   for ib in range(2)]
            b0 = 0
            r0 = 0
            for c, cmb in enumerate(sched):
                # f1 tile and f2 tile are adjacent in the packed buffer;
                # the two HWDGE rings (SP / ACT) stream them concurrently.
                r1 = fv[r0:r0 + cmb * P]
                r2 = fv[r0 + cmb * P:r0 + 2 * cmb * P]
                tb = poolf.tile([P, 2, mb, N], cdt, tag="tf",
                                name=f"tf_{c}")[:, :, :cmb]
                nc.sync.dma_start(
                    out=tb[:, 0], in_=r1.rearrange("(p nb) i -> p nb i", p=P))
                nc.scalar.dma_start(
                    out=tb[:, 1], in_=r2.rearrange("(p nb) i -> p nb i", p=P))
                last_chunk = c == len(sched) - 1
                if not last_chunk:
                    for nb in range(0, cmb, 2):
                        gb = b0 + nb
                        for ib in range(2):
                            nc.tensor.matmul(
                                acc[ib],
                                tb[:, 0, nb:nb + 2, ib * P:(ib + 1) * P],
                                tb[:, 1, nb:nb + 2, :],
                                start=(gb == 0),
                                stop=False,
                                perf_mode=DR,
                            )
                else:
                    # ib-major in the last chunk: acc[0] finishes first so
                    # its PSUM copy + store overlap acc[1]'s final matmuls.
                    for ib in range(2):
                        for nb in range(0, cmb, 2):
                            nc.tensor.matmul(
                                acc[ib],
                                tb[:, 0, nb:nb + 2, ib * P:(ib + 1) * P],
                                tb[:, 1, nb:nb + 2, :],
                                start=False,
                                stop=(nb == cmb - 2),
                                perf_mode=DR,
                            )
                        o = outpool.tile([P, N], mybir.dt.float32,
                                         tag=f"o{ib}", name=f"o{ib}")
                        nc.vector.tensor_copy(o, acc[ib])
                        nc.scalar.dma_start(
                            out=out.ap()[ib * P:(ib + 1) * P, :], in_=o)
                b0 += cmb
                r0 += 2 * cmb * P

    nc.compile()
    return nc


def _get_nc():
    if "nc" not in _built:
        _built["nc"] = _build()
    return _built["nc"]


def _gram_partials(in_maps, trace=False):
    global _last_results
    _install_ntff_hook()
    from concourse.bass_utils import run_bass_kernel_spmd

    nc = _get_nc()
    res = run_bass_kernel_spmd(nc, in_maps, core_ids=list(range(NCORES)),
                               trace=trace)
    _last_results = res
    return [r["out"] for r in res.results]


def _pack_core(f1, f2, k, dloc=DLOC, mb=MB):
    """Pack core k's d-chunks of f1, f2 [N, D] fp32 into one [2*dloc, N]
    fp8e4m3 buffer.

    Chunk c covering blocks [b0, b0+cmb): the f1 tile occupies rows
    [2*b0*P, (2*b0+cmb)*P) and the f2 tile the next cmb*P rows, each in
    [P, cmb, N] SBUF tile order (row p*cmb+nb holds f[:, (b0+nb)*P+p]).
    """
    f8 = _f8dt()
    x1 = np.ascontiguousarray(f1[:, k * dloc:(k + 1) * dloc]).astype(f8)
    x2 = np.ascontiguousarray(f2[:, k * dloc:(k + 1) * dloc]).astype(f8)
    sched = _sched(dloc, mb)
    outbuf = np.empty((2 * dloc, N), dtype=f8)
    b0 = 0
    r0 = 0
    for cmb in sched:
        for x in (x1, x2):
            sl = x[:, b0 * P:(b0 + cmb) * P]              # [N, cmb*P]
            t = sl.reshape(N, cmb, P).transpose(2, 1, 0)  # [P, cmb, N]
            outbuf[r0:r0 + cmb * P] = t.reshape(cmb * P, N)
            r0 += cmb * P
        b0 += cmb
    return {"ft0": outbuf}


def kernel(V1, V2):
    V1 = np.asarray(V1, dtype=np.float32)
    V2 = np.asarray(V2, dtype=np.float32)
    f1 = V1.reshape(N, D)
    f2 = V2.reshape(N, D)

    from concurrent.futures import ThreadPoolExecutor
    with ThreadPoolExecutor(NCORES) as ex:
        in_maps = list(ex.map(lambda k: _pack_core(f1, f2, k), range(NCORES)))
    partials = _gram_partials(in_maps)

    Z = np.zeros((N, N), dtype=np.float64)
    for pmat in partials:
        Z += pmat
    Z /= D

    eps = 1e-12
    p = 1.0 / (1.0 + np.exp(-Z))
    p = np.clip(p, eps, 1.0 - eps)
    lab = np.eye(N, dtype=np.float64)
    loss = -np.mean(lab * np.log(p) + (1.0 - lab) * np.log1p(-p))
    return np.array(loss, dtype=np.float32)


def _selftest_sim():
    """Scaled-down correctness check in CoreSim (no hardware)."""
    from concourse.bass_interp import CoreSim

    dloc, mb = 2048, 4
    nc = _build(dloc=dloc, mb=mb)
    rng = np.random.default_rng(0)
    a = rng.standard_normal((N, dloc)).astype(np.float32)  # [N, dloc] like f1
    b = rng.standard_normal((N, dloc)).astype(np.float32)

    sim = CoreSim(nc)
    for name, arr in _pack_core(a, b, 0, dloc=dloc, mb=mb).items():
        sim.tensor(name)[:] = arr
    sim.simulate()
    got = np.array(sim.tensor("out"))
    f8 = _f8dt()
    aq = a.astype(f8).astype(np.float64)
    bq = b.astype(f8).astype(np.float64)
    want_q = aq @ bq.T
    want = a.astype(np.float64) @ b.astype(np.float64).T
    err_q = np.abs(got - want_q).max() / np.abs(want_q).max()
    err = np.abs(got - want).max() / np.abs(want).max()
    print("selftest rel err vs fp8-exact:", err_q, " vs fp32-exact:", err)
    assert err_q < 1e-5, err_q
    # fp8 quantization noise on individual Gram entries at this small
    # dloc; the final scalar loss at full D is ~1e-8 relative.
    assert err < 6e-2, err
    print("SELFTEST PASSED")


if __name__ == "__main__":
    _selftest_sim()


# revision 8
# speedup vs baseline: 1.1266x; 1.0296x over previous
"""CPC spatial BCE loss kernel for 8 TRN2 NeuronCores.

Computation: loss = BCE(sigmoid((V1.reshape(N,D) @ V2.reshape(N,D).T) / D), eye(N))
with N=256, D=64*64*64=262144.

Strategy (memory-regime): shard the contraction dim D across the 8 cores
and quantize to fp8e4m3 on the host, so each core streams only 16.8 MB
(the 8 cores together saturate the chip's HBM, so bytes-moved is the
whole game; fp8 quantization error on the scalar loss is ~1e-8 relative,
vs the 2e-2 gate). Each core computes a partial Gram matrix [256, 256]
over its 32768-wide slice of D via fp8 DoubleRow TensorE matmuls
(2 fp8 weights/PE cell = 2x ALU rate, keeping TensorE well under the DMA
cadence) accumulated in fp32 PSUM. The host lays out each core's chunk
d-major and pre-tiled to the exact SBUF tile layout, so every chunk DMA
is one fully-contiguous read with the contraction dim landing on SBUF
partitions -- no on-device transposes or casts.

Device pipeline per chunk: fp8 loads stream on the two HWDGE rings (SP
ring for the f1 tile, ACT ring for the adjacent f2 tile of the same
packed buffer -- together they stream at the per-core share of the chip
HBM ceiling); TensorE consumes the tiles directly with DoubleRow
matmuls (lhsT [128d, 2, 128i], rhs [128d, 2, 256j] -> psum [128i, 256j],
two d-blocks per instruction). Chunk sizes are small at the head (fast
pipeline fill) and tail (short drain after the last DMA).

The partial Gram matrices are summed on the host (the unshard step for a
sum-sharded value) and the final sigmoid+BCE over 256x256 values is a
negligible epilogue done in numpy.
"""

import numpy as np

N = 256
D = 64 * 64 * 64  # 262144
NCORES = 8
DLOC = D // NCORES  # 32768
P = 128  # SBUF partitions
MB = 8  # max d-blocks of 128 per DMA chunk

_built = {}
_last_results = None  # test harness reads profiling info from here


def _install_ntff_hook():
    """Best-effort shim: some images lack antenv.axon_hooks, which
    bass_utils imports when profiling is requested (BASS_TRACE) under
    axon. Provide it + register the ctypes NTFF hook so tracing works;
    degrade silently if any piece is missing."""
    import sys
    import types
    try:
        import antenv.axon_hooks  # noqa: F401
        return
    except Exception:
        pass
    try:
        import antenv
        mod = types.ModuleType("antenv.axon_hooks")
        mod._hook = None

        def set_axon_ntff_profile_hook(h, _mod=mod):
            _mod._hook = h

        def get_axon_ntff_profile_hook(_mod=mod):
            return _mod._hook

        mod.set_axon_ntff_profile_hook = set_axon_ntff_profile_hook
        mod.get_axon_ntff_profile_hook = get_axon_ntff_profile_hook
        sys.modules["antenv.axon_hooks"] = mod
        antenv.axon_hooks = mod
    except Exception:
        return
    try:
        # with just the stub registered, bass_utils degrades gracefully
        # (no trace); with the real hook it can profile
        from trn_agent_boot.trn_boot import _ntff_profile_via_ctypes
        hook = _ntff_profile_via_ctypes("/opt/axon/libaxon_pjrt.so")
        if hook is not None:
            set_axon_ntff_profile_hook(hook)
    except Exception:
        pass


def _f8dt():
    import ml_dtypes
    return np.dtype(ml_dtypes.float8_e4m3)


def _sched(dloc=DLOC, mb=MB):
    """Chunk schedule in 128-d blocks: small head (fill) / tail (drain).

    All chunk sizes are even: DoubleRow consumes d-blocks in pairs and a
    pair may not span a chunk boundary.
    """
    nblocks = dloc // P
    assert nblocks % 2 == 0
    if nblocks <= 4 * mb:
        sched = []
        rem = nblocks
        while rem > 0:
            s = min(4, rem)
            sched.append(s)
            rem -= s
        assert all(s % 2 == 0 for s in sched), sched
        return sched
    # Small head chunks fill the pipeline fast. The tail does NOT taper:
    # full-size chunks keep TensorE busy through the drain (idle tapers
    # invite a HAM re-throttle right when the final matmuls run); only
    # the very last chunk is tiny so the DMA->matmul->store chain after
    # the last byte is short.
    sched = [2, 2, 4, 8]
    rem = nblocks - sum(sched) - 2
    while rem >= mb:
        sched.append(mb)
        rem -= mb
    if rem:
        sched.append(rem)
    sched.append(2)
    assert sum(sched) == nblocks, (sched, nblocks)
    assert all(s % 2 == 0 for s in sched), sched
    return sched


def _build(dloc=DLOC, mb=MB, bufs=16):
    """Build + bacc-compile the per-core Bass kernel.

    Per-core input ft0: [2*dloc, N] fp8e4m3, host pre-tiled so chunk c
    (covering blocks [b0, b0+cmb) of 128 d-values) holds its f1 tile and
    f2 tile adjacently, each in [P, cmb, N] SBUF tile order (row
    r = p*cmb + nb holds f[:, (b0+nb)*P + p]) -- i.e. each chunk DMA is
    one contiguous read mapping partition p <- d within block.
    Output: out[i, j] = sum_d f1t[d, i] * f2t[d, j]   (partial Gram)
    """
    import concourse.mybir as mybir
    from concourse import bacc
    from concourse.bass import MemorySpace
    from concourse.tile import TileContext

    cdt = mybir.dt.float8e4
    DR = mybir.MatmulPerfMode.DoubleRow

    nc = bacc.Bacc("TRN2", target_bir_lowering=False, debug=False,
                   num_devices=NCORES)
    # Both tensors packed chunk-interleaved [f1_c | f2_c | f1_c+1 ...] so the
    # two concurrent ring reads hit adjacent address regions (uniform HBM
    # channel striping instead of two far-apart colliding streams).
    sched = _sched(dloc, mb)
    ft = nc.dram_tensor("ft0", (2 * dloc, N), cdt, kind="ExternalInput")
    out = nc.dram_tensor("out", (N, N), mybir.dt.float32,
                         kind="ExternalOutput")

    fv = ft.ap()

    with TileContext(nc) as tc:
        with tc.tile_pool(name="psum", bufs=1, space=MemorySpace.PSUM) as psum_pool, \
             tc.tile_pool(name="sbuff", bufs=bufs) as poolf, \
             tc.tile_pool(name="outp", bufs=1) as outpool:
            acc = [psum_pool.tile([P, N], mybir.dt.float32, tag=f"acc{ib}",
                                  name=f"acc{ib}")
                   for ib in range(2)]
            b0 = 0
            r0 = 0
            for c, cmb in enumerate(sched):
                # f1 tile and f2 tile are adjacent in the packed buffer;
                # the two HWDGE rings (SP / ACT) stream them concurrently.
                r1 = fv[r0:r0 + cmb * P]
                r2 = fv[r0 + cmb * P:r0 + 2 * cmb * P]
                tb = poolf.tile([P, 2, mb, N], cdt, tag="tf",
                                name=f"tf_{c}")[:, :, :cmb]
                nc.sync.dma_start(
                    out=tb[:, 0], in_=r1.rearrange("(p nb) i -> p nb i", p=P))
                nc.scalar.dma_start(
                    out=tb[:, 1], in_=r2.rearrange("(p nb) i -> p nb i", p=P))
                last_chunk = c == len(sched) - 1
                if not last_chunk:
                    for nb in range(0, cmb, 2):
                        gb = b0 + nb
                        for ib in range(2):
                            nc.tensor.matmul(
                                acc[ib],
                                tb[:, 0, nb:nb + 2, ib * P:(ib + 1) * P],
                                tb[:, 1, nb:nb + 2, :],
                                start=(gb == 0),
                                stop=False,
                                perf_mode=DR,
                            )
                else:
                    # ib-major in the last chunk: acc[0] finishes first so
                    # its PSUM copy + store overlap acc[1]'s final matmuls.
                    for ib in range(2):
                        for nb in range(0, cmb, 2):
                            nc.tensor.matmul(
                                acc[ib],
                                tb[:, 0, nb:nb + 2, ib * P:(ib + 1) * P],
                                tb[:, 1, nb:nb + 2, :],
                                start=False,
                                stop=(nb == cmb - 2),
                                perf_mode=DR,
                            )
                        o = outpool.tile([P, N], mybir.dt.float32,
                                         tag=f"o{ib}", name=f"o{ib}")
                        nc.vector.tensor_copy(o, acc[ib])
                        # one store per ring so the two run in parallel
                        eng = nc.sync if ib == 0 else nc.scalar
                        eng.dma_start(
                            out=out.ap()[ib * P:(ib + 1) * P, :], in_=o)
                b0 += cmb
                r0 += 2 * cmb * P

    nc.compile()
    return nc


def _get_nc():
    if "nc" not in _built:
        _built["nc"] = _build()
    return _built["nc"]


def _gram_partials(in_maps, trace=False):
    global _last_results
    _install_ntff_hook()
    from concourse.bass_utils import run_bass_kernel_spmd

    nc = _get_nc()
    res = run_bass_kernel_spmd(nc, in_maps, core_ids=list(range(NCORES)),
                               trace=trace)
    _last_results = res
    return [r["out"] for r in res.results]


def _pack_core(f1, f2, k, dloc=DLOC, mb=MB):
    """Pack core k's d-chunks of f1, f2 [N, D] fp32 into one [2*dloc, N]
    fp8e4m3 buffer.

    Chunk c covering blocks [b0, b0+cmb): the f1 tile occupies rows
    [2*b0*P, (2*b0+cmb)*P) and the f2 tile the next cmb*P rows, each in
    [P, cmb, N] SBUF tile order (row p*cmb+nb holds f[:, (b0+nb)*P+p]).
    """
    f8 = _f8dt()
    x1 = np.ascontiguousarray(f1[:, k * dloc:(k + 1) * dloc]).astype(f8)
    x2 = np.ascontiguousarray(f2[:, k * dloc:(k + 1) * dloc]).astype(f8)
    sched = _sched(dloc, mb)
    outbuf = np.empty((2 * dloc, N), dtype=f8)
    b0 = 0
    r0 = 0
    for cmb in sched:
        for x in (x1, x2):
            sl = x[:, b0 * P:(b0 + cmb) * P]              # [N, cmb*P]
            t = sl.reshape(N, cmb, P).transpose(2, 1, 0)  # [P, cmb, N]
            outbuf[r0:r0 + cmb * P] = t.reshape(cmb * P, N)
            r0 += cmb * P
        b0 += cmb
    return {"ft0": outbuf}


def kernel(V1, V2):
    V1 = np.asarray(V1, dtype=np.float32)
    V2 = np.asarray(V2, dtype=np.float32)
    f1 = V1.reshape(N, D)
    f2 = V2.reshape(N, D)

    from concurrent.futures import ThreadPoolExecutor
    with ThreadPoolExecutor(NCORES) as ex:
        in_maps = list(ex.map(lambda k: _pack_core(f1, f2, k), range(NCORES)))
    partials = _gram_partials(in_maps)

    Z = np.zeros((N, N), dtype=np.float64)
    for pmat in partials:
        Z += pmat
    Z /= D

    eps = 1e-12
    p = 1.0 / (1.0 + np.exp(-Z))
    p = np.clip(p, eps, 1.0 - eps)
    lab = np.eye(N, dtype=np.float64)
    loss = -np.mean(lab * np.log(p) + (1.0 - lab) * np.log1p(-p))
    return np.array(loss, dtype=np.float32)


def _selftest_sim():
    """Scaled-down correctness check in CoreSim (no hardware)."""
    from concourse.bass_interp import CoreSim

    dloc, mb = 2048, 4
    nc = _build(dloc=dloc, mb=mb)
    rng = np.random.default_rng(0)
    a = rng.standard_normal((N, dloc)).astype(np.float32)  # [N, dloc] like f1
    b = rng.standard_normal((N, dloc)).astype(np.float32)

    sim = CoreSim(nc)
    for name, arr in _pack_core(a, b, 0, dloc=dloc, mb=mb).items():
        sim.tensor(name)[:] = arr
    sim.simulate()
    got = np.array(sim.tensor("out"))
    f8 = _f8dt()
    aq = a.astype(f8).astype(np.float64)
    bq = b.astype(f8).astype(np.float64)
    want_q = aq @ bq.T
    want = a.astype(np.float64) @ b.astype(np.float64).T
    err_q = np.abs(got - want_q).max() / np.abs(want_q).max()
    err = np.abs(got - want).max() / np.abs(want).max()
    print("selftest rel err vs fp8-exact:", err_q, " vs fp32-exact:", err)
    assert err_q < 1e-5, err_q
    # fp8 quantization noise on individual Gram entries at this small
    # dloc; the final scalar loss at full D is ~1e-8 relative.
    assert err < 6e-2, err
    print("SELFTEST PASSED")


if __name__ == "__main__":
    _selftest_sim()


# revision 9
# speedup vs baseline: 1.1669x; 1.0358x over previous
"""CPC spatial BCE loss kernel for 8 TRN2 NeuronCores.

Computation: loss = BCE(sigmoid((V1.reshape(N,D) @ V2.reshape(N,D).T) / D), eye(N))
with N=256, D=64*64*64=262144.

Strategy (memory-regime): shard the contraction dim D across the 8 cores
and quantize to fp8e4m3 on the host, so each core streams only 16.8 MB
(the 8 cores together saturate the chip's HBM, so bytes-moved is the
whole game; fp8 quantization error on the scalar loss is ~1e-8 relative,
vs the 2e-2 gate). Each core computes a partial Gram matrix [256, 256]
over its 32768-wide slice of D via fp8 DoubleRow TensorE matmuls
(2 fp8 weights/PE cell = 2x ALU rate, keeping TensorE well under the DMA
cadence) accumulated in fp32 PSUM. The host lays out each core's chunk
d-major and pre-tiled to the exact SBUF tile layout, so every chunk DMA
is one fully-contiguous read with the contraction dim landing on SBUF
partitions -- no on-device transposes or casts.

Device pipeline per chunk: fp8 loads stream on the two HWDGE rings (SP
ring for the f1 tile, ACT ring for the adjacent f2 tile of the same
packed buffer -- together they stream at the per-core share of the chip
HBM ceiling); TensorE consumes the tiles directly with DoubleRow
matmuls (lhsT [128d, 2, 128i], rhs [128d, 2, 256j] -> psum [128i, 256j],
two d-blocks per instruction). Chunk sizes are small at the head (fast
pipeline fill) and tail (short drain after the last DMA).

The partial Gram matrices are summed on the host (the unshard step for a
sum-sharded value) and the final sigmoid+BCE over 256x256 values is a
negligible epilogue done in numpy.
"""

import numpy as np

N = 256
D = 64 * 64 * 64  # 262144
NCORES = 8
DLOC = D // NCORES  # 32768
P = 128  # SBUF partitions
MB = 16  # max d-blocks of 128 per DMA chunk

_built = {}
_last_results = None  # test harness reads profiling info from here


def _install_ntff_hook():
    """Best-effort shim: some images lack antenv.axon_hooks, which
    bass_utils imports when profiling is requested (BASS_TRACE) under
    axon. Provide it + register the ctypes NTFF hook so tracing works;
    degrade silently if any piece is missing."""
    import sys
    import types
    try:
        import antenv.axon_hooks  # noqa: F401
        return
    except Exception:
        pass
    try:
        import antenv
        mod = types.ModuleType("antenv.axon_hooks")
        mod._hook = None

        def set_axon_ntff_profile_hook(h, _mod=mod):
            _mod._hook = h

        def get_axon_ntff_profile_hook(_mod=mod):
            return _mod._hook

        mod.set_axon_ntff_profile_hook = set_axon_ntff_profile_hook
        mod.get_axon_ntff_profile_hook = get_axon_ntff_profile_hook
        sys.modules["antenv.axon_hooks"] = mod
        antenv.axon_hooks = mod
    except Exception:
        return
    try:
        # with just the stub registered, bass_utils degrades gracefully
        # (no trace); with the real hook it can profile
        from trn_agent_boot.trn_boot import _ntff_profile_via_ctypes
        hook = _ntff_profile_via_ctypes("/opt/axon/libaxon_pjrt.so")
        if hook is not None:
            set_axon_ntff_profile_hook(hook)
    except Exception:
        pass


def _f8dt():
    import ml_dtypes
    return np.dtype(ml_dtypes.float8_e4m3)


def _sched(dloc=DLOC, mb=MB):
    """Chunk schedule in 128-d blocks: small head (fill) / tail (drain).

    All chunk sizes are even: DoubleRow consumes d-blocks in pairs and a
    pair may not span a chunk boundary.
    """
    nblocks = dloc // P
    assert nblocks % 2 == 0
    if nblocks <= 4 * mb:
        sched = []
        rem = nblocks
        while rem > 0:
            s = min(4, rem)
            sched.append(s)
            rem -= s
        assert all(s % 2 == 0 for s in sched), sched
        return sched
    # Small head chunks fill the pipeline fast. The tail does NOT taper:
    # full-size chunks keep TensorE busy through the drain (idle tapers
    # invite a HAM re-throttle right when the final matmuls run); only
    # the very last chunk is tiny so the DMA->matmul->store chain after
    # the last byte is short.
    sched = [2, 2, 4, 8]
    rem = nblocks - sum(sched) - 2
    while rem >= mb:
        sched.append(mb)
        rem -= mb
    if rem:
        sched.append(rem)
    sched.append(2)
    assert sum(sched) == nblocks, (sched, nblocks)
    assert all(s % 2 == 0 for s in sched), sched
    return sched


def _build(dloc=DLOC, mb=MB, bufs=8):
    """Build + bacc-compile the per-core Bass kernel.

    Per-core input ft0: [2*dloc, N] fp8e4m3, host pre-tiled so chunk c
    (covering blocks [b0, b0+cmb) of 128 d-values) holds its f1 tile and
    f2 tile adjacently, each in [P, cmb, N] SBUF tile order (row
    r = p*cmb + nb holds f[:, (b0+nb)*P + p]) -- i.e. each chunk DMA is
    one contiguous read mapping partition p <- d within block.
    Output: out[i, j] = sum_d f1t[d, i] * f2t[d, j]   (partial Gram)
    """
    import concourse.mybir as mybir
    from concourse import bacc
    from concourse.bass import MemorySpace
    from concourse.tile import TileContext

    cdt = mybir.dt.float8e4
    DR = mybir.MatmulPerfMode.DoubleRow

    nc = bacc.Bacc("TRN2", target_bir_lowering=False, debug=False,
                   num_devices=NCORES)
    # Both tensors packed chunk-interleaved [f1_c | f2_c | f1_c+1 ...] so the
    # two concurrent ring reads hit adjacent address regions (uniform HBM
    # channel striping instead of two far-apart colliding streams).
    sched = _sched(dloc, mb)
    ft = nc.dram_tensor("ft0", (2 * dloc, N), cdt, kind="ExternalInput")
    out = nc.dram_tensor("out", (N, N), mybir.dt.float32,
                         kind="ExternalOutput")

    fv = ft.ap()

    with TileContext(nc) as tc:
        with tc.tile_pool(name="psum", bufs=1, space=MemorySpace.PSUM) as psum_pool, \
             tc.tile_pool(name="sbuff", bufs=bufs) as poolf, \
             tc.tile_pool(name="outp", bufs=1) as outpool:
            acc = [psum_pool.tile([P, N], mybir.dt.float32, tag=f"acc{ib}",
                                  name=f"acc{ib}")
                   for ib in range(2)]
            b0 = 0
            r0 = 0
            for c, cmb in enumerate(sched):
                # f1 tile and f2 tile are adjacent in the packed buffer;
                # the two HWDGE rings (SP / ACT) stream them concurrently.
                r1 = fv[r0:r0 + cmb * P]
                r2 = fv[r0 + cmb * P:r0 + 2 * cmb * P]
                tb = poolf.tile([P, 2, mb, N], cdt, tag="tf",
                                name=f"tf_{c}")[:, :, :cmb]
                nc.sync.dma_start(
                    out=tb[:, 0], in_=r1.rearrange("(p nb) i -> p nb i", p=P))
                nc.scalar.dma_start(
                    out=tb[:, 1], in_=r2.rearrange("(p nb) i -> p nb i", p=P))
                last_chunk = c == len(sched) - 1
                if not last_chunk:
                    for nb in range(0, cmb, 2):
                        gb = b0 + nb
                        for ib in range(2):
                            nc.tensor.matmul(
                                acc[ib],
                                tb[:, 0, nb:nb + 2, ib * P:(ib + 1) * P],
                                tb[:, 1, nb:nb + 2, :],
                                start=(gb == 0),
                                stop=False,
                                perf_mode=DR,
                            )
                else:
                    # ib-major in the last chunk: acc[0] finishes first so
                    # its PSUM copy + store overlap acc[1]'s final matmuls.
                    for ib in range(2):
                        for nb in range(0, cmb, 2):
                            nc.tensor.matmul(
                                acc[ib],
                                tb[:, 0, nb:nb + 2, ib * P:(ib + 1) * P],
                                tb[:, 1, nb:nb + 2, :],
                                start=False,
                                stop=(nb == cmb - 2),
                                perf_mode=DR,
                            )
                        o = outpool.tile([P, N], mybir.dt.float32,
                                         tag=f"o{ib}", name=f"o{ib}")
                        nc.vector.tensor_copy(o, acc[ib])
                        # one store per ring so the two run in parallel
                        eng = nc.sync if ib == 0 else nc.scalar
                        eng.dma_start(
                            out=out.ap()[ib * P:(ib + 1) * P, :], in_=o)
                b0 += cmb
                r0 += 2 * cmb * P

    nc.compile()
    return nc


def _get_nc():
    if "nc" not in _built:
        _built["nc"] = _build()
    return _built["nc"]


def _gram_partials(in_maps, trace=False):
    global _last_results
    _install_ntff_hook()
    from concourse.bass_utils import run_bass_kernel_spmd

    nc = _get_nc()
    res = run_bass_kernel_spmd(nc, in_maps, core_ids=list(range(NCORES)),
                               trace=trace)
    _last_results = res
    return [r["out"] for r in res.results]


def _pack_core(f1, f2, k, dloc=DLOC, mb=MB):
    """Pack core k's d-chunks of f1, f2 [N, D] fp32 into one [2*dloc, N]
    fp8e4m3 buffer.

    Chunk c covering blocks [b0, b0+cmb): the f1 tile occupies rows
    [2*b0*P, (2*b0+cmb)*P) and the f2 tile the next cmb*P rows, each in
    [P, cmb, N] SBUF tile order (row p*cmb+nb holds f[:, (b0+nb)*P+p]).
    """
    f8 = _f8dt()
    x1 = np.ascontiguousarray(f1[:, k * dloc:(k + 1) * dloc]).astype(f8)
    x2 = np.ascontiguousarray(f2[:, k * dloc:(k + 1) * dloc]).astype(f8)
    sched = _sched(dloc, mb)
    outbuf = np.empty((2 * dloc, N), dtype=f8)
    b0 = 0
    r0 = 0
    for cmb in sched:
        for x in (x1, x2):
            sl = x[:, b0 * P:(b0 + cmb) * P]              # [N, cmb*P]
            t = sl.reshape(N, cmb, P).transpose(2, 1, 0)  # [P, cmb, N]
            outbuf[r0:r0 + cmb * P] = t.reshape(cmb * P, N)
            r0 += cmb * P
        b0 += cmb
    return {"ft0": outbuf}


def kernel(V1, V2):
    V1 = np.asarray(V1, dtype=np.float32)
    V2 = np.asarray(V2, dtype=np.float32)
    f1 = V1.reshape(N, D)
    f2 = V2.reshape(N, D)

    from concurrent.futures import ThreadPoolExecutor
    with ThreadPoolExecutor(NCORES) as ex:
        in_maps = list(ex.map(lambda k: _pack_core(f1, f2, k), range(NCORES)))
    partials = _gram_partials(in_maps)

    Z = np.zeros((N, N), dtype=np.float64)
    for pmat in partials:
        Z += pmat
    Z /= D

    eps = 1e-12
    p = 1.0 / (1.0 + np.exp(-Z))
    p = np.clip(p, eps, 1.0 - eps)
    lab = np.eye(N, dtype=np.float64)
    loss = -np.mean(lab * np.log(p) + (1.0 - lab) * np.log1p(-p))
    return np.array(loss, dtype=np.float32)


def _selftest_sim():
    """Scaled-down correctness check in CoreSim (no hardware)."""
    from concourse.bass_interp import CoreSim

    dloc, mb = 2048, 4
    nc = _build(dloc=dloc, mb=mb)
    rng = np.random.default_rng(0)
    a = rng.standard_normal((N, dloc)).astype(np.float32)  # [N, dloc] like f1
    b = rng.standard_normal((N, dloc)).astype(np.float32)

    sim = CoreSim(nc)
    for name, arr in _pack_core(a, b, 0, dloc=dloc, mb=mb).items():
        sim.tensor(name)[:] = arr
    sim.simulate()
    got = np.array(sim.tensor("out"))
    f8 = _f8dt()
    aq = a.astype(f8).astype(np.float64)
    bq = b.astype(f8).astype(np.float64)
    want_q = aq @ bq.T
    want = a.astype(np.float64) @ b.astype(np.float64).T
    err_q = np.abs(got - want_q).max() / np.abs(want_q).max()
    err = np.abs(got - want).max() / np.abs(want).max()
    print("selftest rel err vs fp8-exact:", err_q, " vs fp32-exact:", err)
    assert err_q < 1e-5, err_q
    # fp8 quantization noise on individual Gram entries at this small
    # dloc; the final scalar loss at full D is ~1e-8 relative.
    assert err < 6e-2, err
    print("SELFTEST PASSED")


if __name__ == "__main__":
    _selftest_sim()
